# revision 35
# baseline (speedup 1.0000x reference)
"""Block-sparse linear y = x @ W^T on 8 Trainium2 NeuronCores.

Strategy: the 32x32 block structure (50% block density, random scatter) is not
exploitable on a 128x128 PE array (any packing at 32-granularity wastes more
PE volume than the ~39% merged density saves), so we densify W^T on the host
(cheap: 8MB of scatter-adds) and run a dense GEMM, sharded 4-way over tokens
x 2-way over out_features (8 cores, no collectives).

The matmuls run in bfloat16 (PE rate is identical to float32r at 1 output
column/cycle, so a pure-bf16 GEMM costs 131072 cycles = 54.6us/core), which
halves input HBM traffic vs fp32r and takes the DMA stream off the PE's
critical path — the fp32r version sat exactly at the 20MB/55us ridge and any
DMA jitter stalled the PE. On top of that, the LAST 256 K-columns run as a
single fp8(e4m3) DoubleRow matmul per psum pass: DoubleRow packs the K-pair
into one partition slot (2 MACs/cell), so one 216ns matmul contracts 256 K —
replacing two bf16 k-steps and cutting PE time ~6% to 51.9us/core. Error is
dominated by the raw-fp8 segment and measures 1.49e-2 max-rel vs the fp32
reference on this data (tolerance 2e-2; pure bf16 is 2.0e-3, and extending
fp8 to 512 K measures 1.85e-2 — too close to the gate). fp8 across ALL of K
single-pass would be 2x faster but fails accuracy (5.1e-2), and hi/lo
multi-term fp8 splits cost 1.5x bf16 since DoubleRow does not raise the
column rate — per-instruction K-depth is the only fp8 win.

Schedule per core, two passes over the out-feature halves:
(1) n=0, k-outer/m-inner. Each k-step's x tile and W tile are host-packed
    into ONE bundle = one linear DMA = one completion semaphore, so a k-step
    becomes ready atomically (per-DMA completion order jitters +-1.5us, so
    splitting a k-step across DMAs stalls the PE mid-step and drops the HAM
    clock).
(2) n=1, m-outer/k-inner: everything is SBUF-resident by now (the n=1 W
    prefetches behind the n=0 stream), so each bank runs its 15 matmuls
    (14 bf16 + 1 fp8-DR) back-to-back and drains (copy + store) while the
    next bank computes. The last bank accumulates as two 256-col half-psums
    in different banks (start=True zeroes a whole 2KB bank region) so its
    first half drains during the second half's matmuls, and the two half
    stores ride different engines/queues — the post-last-matmul tail is one
    256-col copy+store.
All input DMAs ride the sync-engine queue in consumption order; y stores ride
the scalar-engine queue so stores never delay loads. Short warmup matmuls
(tiny memset dependency) keep the PE busy from preamble-end until the first
bundle lands, holding the HAM clock gate open so real matmuls start at full
clock.
"""

import numpy as np

TOKENS, IN_F, OUT_F = 4096, 2048, 2048
BLOCK = 32
N_CORES = 8
TG, OG = 4, 2  # token groups x out-feature groups
T_SH = TOKENS // TG  # 1024 tokens per core
O_SH = OUT_F // OG  # 1024 out features per core
P = 128
NFREE = 512  # PSUM bank free dim (fp32)
KT = IN_F // P  # 16 k tiles
KB = 14  # k-tiles computed in bf16; the last 2 run as one fp8 DoubleRow step
MT = T_SH // P  # 8 psum banks
XH = T_SH // 2  # token half (k=0/k=1 head bundles)
N_WARM = 26  # PE clock-gate warmup matmuls (~150ns each, sized to DMA head)

MM_DTYPE = "bfloat16"  # "bfloat16" (fast DMA) or "float32r" (exact-ish)
TRACE = False  # set by test.py to capture an NTFF profile

_nc_cache = {}
_last_result = None  # BassKernelResults of the most recent run (for test.py)


def _build_nc():
    import concourse.mybir as mybir
    import concourse.tile as tile
    from concourse import bacc

    key = MM_DTYPE
    if key in _nc_cache:
        return _nc_cache[key]

    dt_mm = getattr(mybir.dt, MM_DTYPE)
    f32 = mybir.dt.float32
    f8 = mybir.dt.float8e4
    DR = mybir.MatmulPerfMode.DoubleRow

    nc = bacc.Bacc(None, target_bir_lowering=False)
    # Host-pre-blocked inputs (exact SBUF layouts; all DMAs are linear):
    # bn: per-k bf16 bundles [KB][P][T_SH + NFREE] = [x^T k-tile | w n0 k-tile]
    # w1: n=1 bf16 W^T supertile [P][KB][NFREE]
    # b8: fp8 K-segment bundle [P][2][T_SH + NFREE] = [x8 | w8 n0], K-pair
    #     slot i = k-tile 14+i (one DoubleRow matmul contracts all 256 K)
    # w81: fp8 K-segment n=1 W [P][2][NFREE]
    bn = nc.dram_tensor("bn", [KB, P, T_SH + NFREE], dt_mm, kind="ExternalInput")
    w1q = nc.dram_tensor("w1q", [P, KB, NFREE], dt_mm, kind="ExternalInput")
    b8q = nc.dram_tensor("b8q", [P, 2, T_SH + NFREE], f8, kind="ExternalInput")
    w81q = nc.dram_tensor("w81q", [P, 2, NFREE], f8, kind="ExternalInput")
    y = nc.dram_tensor("y", [T_SH, O_SH], f32, kind="ExternalOutput")

    with tile.TileContext(nc) as tc:
        with (
            tc.tile_pool(name="xp", bufs=1) as xp,
            tc.tile_pool(name="wp", bufs=1) as wp,
            tc.tile_pool(name="op", bufs=1) as op,
            tc.tile_pool(name="ps", bufs=1, space="PSUM") as ps,
        ):
            # Warm the PE's HAM clock gate during the initial DMA head wait.
            zt = xp.tile([P, P], dt_mm, tag="warm", name="warm")
            nc.gpsimd.memset(zt[:], 0.0)
            warm_ps = ps.tile([P, NFREE], f32, tag="ps0", name="warm_ps")
            for _ in range(N_WARM):
                nc.tensor.matmul(warm_ps[:, :P], zt[:], zt[:], start=True, stop=True)

            bnt = [None] * KT  # bundle tiles [P, T_SH + NFREE]

            def lhsT(m, k):
                """Stationary x^T slice for bank m, k-tile k."""
                return bnt[k][:, m * P : (m + 1) * P]

            def psums():
                return [
                    ps.tile([P, NFREE], f32, tag=f"ps{m}", name=f"ps{m}")
                    for m in range(MT)
                ]

            # ---- Pass 1: n=0, k-outer/m-inner, bundles streamed JIT ----
            ps0 = psums()
            for k in range(KB):
                t = xp.tile([P, T_SH + NFREE], dt_mm, tag=f"bn{k}", name=f"bn{k}")
                nc.sync.dma_start(t[:], bn[k])
                bnt[k] = t
                for m in range(MT):
                    nc.tensor.matmul(
                        ps0[m][:],
                        lhsT(m, k),
                        t[:, T_SH : T_SH + NFREE],
                        start=(k == 0),
                        stop=False,
                    )
            # fp8 K-segment: one DoubleRow matmul contracts k-tiles 14+15
            b8 = xp.tile([P, 2, T_SH + NFREE], f8, tag="b8", name="b8")
            nc.sync.dma_start(b8[:], b8q[:])
            for m in range(MT):
                nc.tensor.matmul(
                    ps0[m][:],
                    b8[:, :, m * P : (m + 1) * P],
                    b8[:, :, T_SH : T_SH + NFREE],
                    start=False,
                    stop=True,
                    perf_mode=DR,
                )

            # n=1 W: two bf16 prefetches + the fp8 segment W, queued behind
            # the n=0 stream
            w1 = []
            for h in range(2):
                wt = wp.tile([P, KB // 2, NFREE], dt_mm, tag=f"w1_{h}", name=f"w1_{h}")
                nc.sync.dma_start(
                    wt[:], w1q[:, h * (KB // 2) : (h + 1) * (KB // 2), :]
                )
                w1.append(wt)
            w81 = wp.tile([P, 2, NFREE], f8, tag="w81", name="w81")
            nc.sync.dma_start(w81[:], w81q[:])

            for m in range(MT):  # evict n=0 psums; y stores on the scalar queue
                ot = op.tile([P, NFREE], f32, tag=f"o0_{m}", name=f"o0_{m}")
                nc.vector.tensor_copy(ot[:], ps0[m][:])
                nc.scalar.dma_start(y[m * P : (m + 1) * P, 0:NFREE], ot[:])

            # ---- Pass 2: n=1, m-outer/k-inner; each bank drains as it ends ----
            ps1 = psums()
            for m in range(MT):
                ot = op.tile([P, NFREE], f32, tag=f"o1_{m}", name=f"o1_{m}")
                if m == MT - 1:
                    # last bank: accumulate as two 256-col half-psums so the
                    # first half drains while the second computes — the
                    # post-last-matmul chain shrinks to a 256-col copy+store.
                    # start=True zeroes a whole 2KB bank region, so half 1
                    # must NOT share half 0's bank: it reuses bank 0 (m=0's
                    # psum, drained ~20us earlier) instead.
                    for h in range(2):
                        acc = ps1[m] if h == 0 else ps1[0]
                        sl = slice(0, NFREE // 2)
                        hsl = slice(h * (NFREE // 2), (h + 1) * (NFREE // 2))
                        for k in range(KB):
                            nc.tensor.matmul(
                                acc[:, sl],
                                lhsT(m, k),
                                w1[k // (KB // 2)][:, k % (KB // 2), hsl],
                                start=(k == 0),
                                stop=False,
                            )
                        nc.tensor.matmul(
                            acc[:, sl],
                            b8[:, :, m * P : (m + 1) * P],
                            w81[:, :, hsl],
                            start=False,
                            stop=True,
                            perf_mode=DR,
                        )
                        osl = slice(h * (NFREE // 2), (h + 1) * (NFREE // 2))
                        (nc.vector.tensor_copy if h == 0 else nc.scalar.copy)(
                            ot[:, osl], acc[:, sl]
                        )
                        (nc.scalar if h == 0 else nc.sync).dma_start(
                            y[
                                m * P : (m + 1) * P,
                                NFREE + h * (NFREE // 2) : NFREE
                                + (h + 1) * (NFREE // 2),
                            ],
                            ot[:, osl],
                        )
                else:
                    for k in range(KB):
                        nc.tensor.matmul(
                            ps1[m][:],
                            lhsT(m, k),
                            w1[k // (KB // 2)][:, k % (KB // 2), :],
                            start=(k == 0),
                            stop=False,
                        )
                    nc.tensor.matmul(
                        ps1[m][:],
                        b8[:, :, m * P : (m + 1) * P],
                        w81[:],
                        start=False,
                        stop=True,
                        perf_mode=DR,
                    )
                    nc.vector.tensor_copy(ot[:], ps1[m][:])
                    nc.scalar.dma_start(
                        y[m * P : (m + 1) * P, NFREE : 2 * NFREE], ot[:]
                    )

    nc.compile()
    _nc_cache[key] = nc
    return nc


def _densify_wT(weight_blocks, block_rows, block_cols):
    """Scatter-add the 32x32 blocks into dense W^T [in_features, out_features]."""
    nc_blk = IN_F // BLOCK
    nr_blk = OUT_F // BLOCK
    wcr = np.zeros((nc_blk, nr_blk, BLOCK, BLOCK), np.float32)
    # block b occupies W[32r:32r+32, 32c:32c+32]; W^T gets the transposed block
    np.add.at(
        wcr,
        (block_cols.astype(np.int64), block_rows.astype(np.int64)),
        np.swapaxes(weight_blocks.astype(np.float32, copy=False), 1, 2),
    )
    return np.ascontiguousarray(wcr.transpose(0, 2, 1, 3).reshape(IN_F, OUT_F))


def _mm_np_dtype():
    if MM_DTYPE == "bfloat16":
        import ml_dtypes

        return np.dtype(ml_dtypes.bfloat16)
    return np.dtype(np.float32)


def _pack_core_inputs(xT_sh, wT_sh):
    """Block one core's x^T and W^T shards into the kernel's DMA layouts."""
    import ml_dtypes

    dt = _mm_np_dtype()
    f8 = np.dtype(ml_dtypes.float8_e4m3)
    X = xT_sh.reshape(KT, P, T_SH)  # [k, p, t] fp32
    W = wT_sh.reshape(KT, P, 2, NFREE).transpose(2, 0, 1, 3)  # [n, k, p, o]
    bn = np.concatenate([X[:KB], W[0, :KB]], axis=2).astype(dt)
    w1 = np.ascontiguousarray(W[1, :KB].transpose(1, 0, 2)).astype(dt)
    # fp8 K-segment (k-tiles 14,15 -> DoubleRow pair slot i, global
    # k = KB*128 + i*128 + p for both operands)
    x8 = X[KB:].astype(f8)  # [i, p, t]
    w8 = W[:, KB:].astype(f8)  # [n, i, p, o]
    b8 = np.concatenate([x8, w8[0]], axis=2).transpose(1, 0, 2)  # [p, i, c]
    w81 = w8[1].transpose(1, 0, 2)  # [p, i, o]
    return {
        "bn": np.ascontiguousarray(bn),
        "w1q": w1,
        "b8q": np.ascontiguousarray(b8),
        "w81q": np.ascontiguousarray(w81),
    }


def kernel(x, weight_blocks, block_rows, block_cols):
    global _last_result
    from concourse.bass_utils import run_bass_kernel_spmd

    x = np.asarray(x, dtype=np.float32)
    wT = _densify_wT(
        np.asarray(weight_blocks), np.asarray(block_rows), np.asarray(block_cols)
    )
    xT = np.ascontiguousarray(x.T)

    in_maps = []
    for c in range(N_CORES):
        tg, og = divmod(c, OG)
        in_maps.append(
            _pack_core_inputs(
                xT[:, tg * T_SH : (tg + 1) * T_SH],
                wT[:, og * O_SH : (og + 1) * O_SH],
            )
        )

    nc = _build_nc()
    res = None
    for attempt in range(3):  # transient NRT device errors happen; retry
        try:
            res = run_bass_kernel_spmd(
                nc, in_maps, core_ids=list(range(N_CORES)), trace=TRACE
            )
            break
        except Exception:
            if attempt == 2:
                raise
            import time

            time.sleep(3)
    _last_result = res

    y = np.empty((TOKENS, OUT_F), np.float32)
    for c in range(N_CORES):
        tg, og = divmod(c, OG)
        y[tg * T_SH : (tg + 1) * T_SH, og * O_SH : (og + 1) * O_SH] = res.results[c][
            "y"
        ]
    return y


# revision 36
# speedup vs baseline: 1.0017x; 1.0017x over previous
"""Block-sparse linear y = x @ W^T on 8 Trainium2 NeuronCores.

Strategy: the 32x32 block structure (50% block density, random scatter) is not
exploitable on a 128x128 PE array (any packing at 32-granularity wastes more
PE volume than the ~39% merged density saves), so we densify W^T on the host
(cheap: 8MB of scatter-adds) and run a dense GEMM, sharded 4-way over tokens
x 2-way over out_features (8 cores, no collectives).

The matmuls run in bfloat16 (PE rate is identical to float32r at 1 output
column/cycle, so a pure-bf16 GEMM costs 131072 cycles = 54.6us/core), which
halves input HBM traffic vs fp32r and takes the DMA stream off the PE's
critical path — the fp32r version sat exactly at the 20MB/55us ridge and any
DMA jitter stalled the PE. On top of that, the LAST 256 K-columns run as a
single fp8(e4m3) DoubleRow matmul per psum pass: DoubleRow packs the K-pair
into one partition slot (2 MACs/cell), so one 216ns matmul contracts 256 K —
replacing two bf16 k-steps and cutting PE time ~6% to 51.9us/core. Error is
dominated by the raw-fp8 segment and measures 1.49e-2 max-rel vs the fp32
reference on this data (tolerance 2e-2; pure bf16 is 2.0e-3, and extending
fp8 to 512 K measures 1.85e-2 — too close to the gate). fp8 across ALL of K
single-pass would be 2x faster but fails accuracy (5.1e-2), and hi/lo
multi-term fp8 splits cost 1.5x bf16 since DoubleRow does not raise the
column rate — per-instruction K-depth is the only fp8 win.

Schedule per core, two passes over the out-feature halves:
(1) n=0, k-outer/m-inner. Each k-step's x tile and W tile are host-packed
    into ONE bundle = one linear DMA = one completion semaphore, so a k-step
    becomes ready atomically (per-DMA completion order jitters +-1.5us, so
    splitting a k-step across DMAs stalls the PE mid-step and drops the HAM
    clock).
(2) n=1, m-outer/k-inner: everything is SBUF-resident by now (the n=1 W
    prefetches behind the n=0 stream), so each bank runs its 15 matmuls
    (14 bf16 + 1 fp8-DR) back-to-back and drains (copy + store) while the
    next bank computes. The last bank accumulates as two 256-col half-psums
    in different banks (start=True zeroes a whole 2KB bank region) so its
    first half drains during the second half's matmuls, and the two half
    stores ride different engines/queues — the post-last-matmul tail is one
    256-col copy+store.
All input DMAs ride the sync-engine queue in consumption order; y stores ride
the scalar-engine queue so stores never delay loads. Short warmup matmuls
(tiny memset dependency) keep the PE busy from preamble-end until the first
bundle lands, holding the HAM clock gate open so real matmuls start at full
clock.
"""

import numpy as np

TOKENS, IN_F, OUT_F = 4096, 2048, 2048
BLOCK = 32
N_CORES = 8
TG, OG = 4, 2  # token groups x out-feature groups
T_SH = TOKENS // TG  # 1024 tokens per core
O_SH = OUT_F // OG  # 1024 out features per core
P = 128
NFREE = 512  # PSUM bank free dim (fp32)
KT = IN_F // P  # 16 k tiles
KB = 14  # k-tiles computed in bf16; the last 2 run as one fp8 DoubleRow step
MT = T_SH // P  # 8 psum banks
XH = T_SH // 2  # token half (k=0/k=1 head bundles)
N_WARM = 26  # PE clock-gate warmup matmuls (~150ns each, sized to DMA head)

MM_DTYPE = "bfloat16"  # "bfloat16" (fast DMA) or "float32r" (exact-ish)
TRACE = False  # set by test.py to capture an NTFF profile

_nc_cache = {}
_last_result = None  # BassKernelResults of the most recent run (for test.py)


def _build_nc():
    import concourse.mybir as mybir
    import concourse.tile as tile
    from concourse import bacc

    key = MM_DTYPE
    if key in _nc_cache:
        return _nc_cache[key]

    dt_mm = getattr(mybir.dt, MM_DTYPE)
    f32 = mybir.dt.float32
    f8 = mybir.dt.float8e4
    DR = mybir.MatmulPerfMode.DoubleRow

    nc = bacc.Bacc(None, target_bir_lowering=False)
    # Host-pre-blocked inputs (exact SBUF layouts; all DMAs are linear):
    # bn: per-k bf16 bundles [KB][P][T_SH + NFREE] = [x^T k-tile | w n0 k-tile]
    # w1: n=1 bf16 W^T supertile [P][KB][NFREE]
    # b8: fp8 K-segment bundle [P][2][T_SH + NFREE] = [x8 | w8 n0], K-pair
    #     slot i = k-tile 14+i (one DoubleRow matmul contracts all 256 K)
    # w81: fp8 K-segment n=1 W [P][2][NFREE]
    bn = nc.dram_tensor("bn", [KB, P, T_SH + NFREE], dt_mm, kind="ExternalInput")
    w1q = nc.dram_tensor("w1q", [P, KB, NFREE], dt_mm, kind="ExternalInput")
    b8q = nc.dram_tensor("b8q", [P, 2, T_SH + NFREE], f8, kind="ExternalInput")
    w81q = nc.dram_tensor("w81q", [P, 2, NFREE], f8, kind="ExternalInput")
    y = nc.dram_tensor("y", [T_SH, O_SH], f32, kind="ExternalOutput")

    with tile.TileContext(nc) as tc:
        with (
            tc.tile_pool(name="xp", bufs=1) as xp,
            tc.tile_pool(name="wp", bufs=1) as wp,
            tc.tile_pool(name="op", bufs=1) as op,
            tc.tile_pool(name="ps", bufs=1, space="PSUM") as ps,
        ):
            # Warm the PE's HAM clock gate during the initial DMA head wait.
            zt = xp.tile([P, P], dt_mm, tag="warm", name="warm")
            nc.gpsimd.memset(zt[:], 0.0)
            warm_ps = ps.tile([P, NFREE], f32, tag="ps0", name="warm_ps")
            for _ in range(N_WARM):
                nc.tensor.matmul(warm_ps[:, :P], zt[:], zt[:], start=True, stop=True)

            bnt = [None] * KT  # bundle tiles [P, T_SH + NFREE]

            def lhsT(m, k):
                """Stationary x^T slice for bank m, k-tile k."""
                return bnt[k][:, m * P : (m + 1) * P]

            def psums():
                return [
                    ps.tile([P, NFREE], f32, tag=f"ps{m}", name=f"ps{m}")
                    for m in range(MT)
                ]

            # ---- Pass 1: n=0, k-outer/m-inner, bundles streamed JIT ----
            ps0 = psums()
            for k in range(KB):
                t = xp.tile([P, T_SH + NFREE], dt_mm, tag=f"bn{k}", name=f"bn{k}")
                nc.sync.dma_start(t[:], bn[k])
                bnt[k] = t
                for m in range(MT):
                    nc.tensor.matmul(
                        ps0[m][:],
                        lhsT(m, k),
                        t[:, T_SH : T_SH + NFREE],
                        start=(k == 0),
                        stop=False,
                    )
            # fp8 K-segment: one DoubleRow matmul contracts k-tiles 14+15
            b8 = xp.tile([P, 2, T_SH + NFREE], f8, tag="b8", name="b8")
            nc.sync.dma_start(b8[:], b8q[:])
            for m in range(MT):
                nc.tensor.matmul(
                    ps0[m][:],
                    b8[:, :, m * P : (m + 1) * P],
                    b8[:, :, T_SH : T_SH + NFREE],
                    start=False,
                    stop=True,
                    perf_mode=DR,
                )

            # n=1 W: two bf16 prefetches + the fp8 segment W, queued behind
            # the n=0 stream
            w1 = []
            for h in range(2):
                wt = wp.tile([P, KB // 2, NFREE], dt_mm, tag=f"w1_{h}", name=f"w1_{h}")
                nc.sync.dma_start(
                    wt[:], w1q[:, h * (KB // 2) : (h + 1) * (KB // 2), :]
                )
                w1.append(wt)
            w81 = wp.tile([P, 2, NFREE], f8, tag="w81", name="w81")
            nc.sync.dma_start(w81[:], w81q[:])

            for m in range(MT):  # evict n=0 psums; y stores on the scalar queue
                ot = op.tile([P, NFREE], f32, tag=f"o0_{m}", name=f"o0_{m}")
                nc.vector.tensor_copy(ot[:], ps0[m][:])
                nc.scalar.dma_start(y[m * P : (m + 1) * P, 0:NFREE], ot[:])

            # ---- Pass 2: n=1, m-outer/k-inner; each bank drains as it ends ----
            ps1 = psums()
            for m in range(MT):
                ot = op.tile([P, NFREE], f32, tag=f"o1_{m}", name=f"o1_{m}")
                if m == MT - 1:
                    # last bank: accumulate as two 256-col half-psums so the
                    # first half drains while the second computes — the
                    # post-last-matmul chain shrinks to a 256-col copy+store.
                    # start=True zeroes a whole 2KB bank region, so half 1
                    # must NOT share half 0's bank: it reuses bank 0 (m=0's
                    # psum, drained ~20us earlier) instead.
                    for h in range(2):
                        acc = ps1[m] if h == 0 else ps1[0]
                        sl = slice(0, NFREE // 2)
                        hsl = slice(h * (NFREE // 2), (h + 1) * (NFREE // 2))
                        for k in range(KB):
                            nc.tensor.matmul(
                                acc[:, sl],
                                lhsT(m, k),
                                w1[k // (KB // 2)][:, k % (KB // 2), hsl],
                                start=(k == 0),
                                stop=False,
                            )
                        nc.tensor.matmul(
                            acc[:, sl],
                            b8[:, :, m * P : (m + 1) * P],
                            w81[:, :, hsl],
                            start=False,
                            stop=True,
                            perf_mode=DR,
                        )
                        osl = slice(h * (NFREE // 2), (h + 1) * (NFREE // 2))
                        # both halves copy on the (otherwise idle) vector
                        # engine — the halves are staggered 1.6us apart, and
                        # DVE's copy+sem path is ~0.2us faster than Act's
                        nc.vector.tensor_copy(ot[:, osl], acc[:, sl])
                        (nc.scalar if h == 0 else nc.sync).dma_start(
                            y[
                                m * P : (m + 1) * P,
                                NFREE + h * (NFREE // 2) : NFREE
                                + (h + 1) * (NFREE // 2),
                            ],
                            ot[:, osl],
                        )
                else:
                    for k in range(KB):
                        nc.tensor.matmul(
                            ps1[m][:],
                            lhsT(m, k),
                            w1[k // (KB // 2)][:, k % (KB // 2), :],
                            start=(k == 0),
                            stop=False,
                        )
                    nc.tensor.matmul(
                        ps1[m][:],
                        b8[:, :, m * P : (m + 1) * P],
                        w81[:],
                        start=False,
                        stop=True,
                        perf_mode=DR,
                    )
                    nc.vector.tensor_copy(ot[:], ps1[m][:])
                    nc.scalar.dma_start(
                        y[m * P : (m + 1) * P, NFREE : 2 * NFREE], ot[:]
                    )

    nc.compile()
    _nc_cache[key] = nc
    return nc


def _densify_wT(weight_blocks, block_rows, block_cols):
    """Scatter-add the 32x32 blocks into dense W^T [in_features, out_features]."""
    nc_blk = IN_F // BLOCK
    nr_blk = OUT_F // BLOCK
    wcr = np.zeros((nc_blk, nr_blk, BLOCK, BLOCK), np.float32)
    # block b occupies W[32r:32r+32, 32c:32c+32]; W^T gets the transposed block
    np.add.at(
        wcr,
        (block_cols.astype(np.int64), block_rows.astype(np.int64)),
        np.swapaxes(weight_blocks.astype(np.float32, copy=False), 1, 2),
    )
    return np.ascontiguousarray(wcr.transpose(0, 2, 1, 3).reshape(IN_F, OUT_F))


def _mm_np_dtype():
    if MM_DTYPE == "bfloat16":
        import ml_dtypes

        return np.dtype(ml_dtypes.bfloat16)
    return np.dtype(np.float32)


def _pack_core_inputs(xT_sh, wT_sh):
    """Block one core's x^T and W^T shards into the kernel's DMA layouts."""
    import ml_dtypes

    dt = _mm_np_dtype()
    f8 = np.dtype(ml_dtypes.float8_e4m3)
    X = xT_sh.reshape(KT, P, T_SH)  # [k, p, t] fp32
    W = wT_sh.reshape(KT, P, 2, NFREE).transpose(2, 0, 1, 3)  # [n, k, p, o]
    bn = np.concatenate([X[:KB], W[0, :KB]], axis=2).astype(dt)
    w1 = np.ascontiguousarray(W[1, :KB].transpose(1, 0, 2)).astype(dt)
    # fp8 K-segment (k-tiles 14,15 -> DoubleRow pair slot i, global
    # k = KB*128 + i*128 + p for both operands)
    x8 = X[KB:].astype(f8)  # [i, p, t]
    w8 = W[:, KB:].astype(f8)  # [n, i, p, o]
    b8 = np.concatenate([x8, w8[0]], axis=2).transpose(1, 0, 2)  # [p, i, c]
    w81 = w8[1].transpose(1, 0, 2)  # [p, i, o]
    return {
        "bn": np.ascontiguousarray(bn),
        "w1q": w1,
        "b8q": np.ascontiguousarray(b8),
        "w81q": np.ascontiguousarray(w81),
    }


def kernel(x, weight_blocks, block_rows, block_cols):
    global _last_result
    from concourse.bass_utils import run_bass_kernel_spmd

    x = np.asarray(x, dtype=np.float32)
    wT = _densify_wT(
        np.asarray(weight_blocks), np.asarray(block_rows), np.asarray(block_cols)
    )
    xT = np.ascontiguousarray(x.T)

    in_maps = []
    for c in range(N_CORES):
        tg, og = divmod(c, OG)
        in_maps.append(
            _pack_core_inputs(
                xT[:, tg * T_SH : (tg + 1) * T_SH],
                wT[:, og * O_SH : (og + 1) * O_SH],
            )
        )

    nc = _build_nc()
    res = None
    for attempt in range(3):  # transient NRT device errors happen; retry
        try:
            res = run_bass_kernel_spmd(
                nc, in_maps, core_ids=list(range(N_CORES)), trace=TRACE
            )
            break
        except Exception:
            if attempt == 2:
                raise
            import time

            time.sleep(3)
    _last_result = res

    y = np.empty((TOKENS, OUT_F), np.float32)
    for c in range(N_CORES):
        tg, og = divmod(c, OG)
        y[tg * T_SH : (tg + 1) * T_SH, og * O_SH : (og + 1) * O_SH] = res.results[c][
            "y"
        ]
    return y


# revision 37
# speedup vs baseline: 1.0037x; 1.0020x over previous
"""Block-sparse linear y = x @ W^T on 8 Trainium2 NeuronCores.

Strategy: the 32x32 block structure (50% block density, random scatter) is not
exploitable on a 128x128 PE array (any packing at 32-granularity wastes more
PE volume than the ~39% merged density saves), so we densify W^T on the host
(cheap: 8MB of scatter-adds) and run a dense GEMM, sharded 4-way over tokens
x 2-way over out_features (8 cores, no collectives).

The matmuls run in bfloat16 (PE rate is identical to float32r at 1 output
column/cycle, so a pure-bf16 GEMM costs 131072 cycles = 54.6us/core), which
halves input HBM traffic vs fp32r and takes the DMA stream off the PE's
critical path — the fp32r version sat exactly at the 20MB/55us ridge and any
DMA jitter stalled the PE. On top of that, the LAST 256 K-columns run as a
single fp8(e4m3) DoubleRow matmul per psum pass: DoubleRow packs the K-pair
into one partition slot (2 MACs/cell), so one 216ns matmul contracts 256 K —
replacing two bf16 k-steps and cutting PE time ~6% to 51.9us/core. Error is
dominated by the raw-fp8 segment and measures 1.49e-2 max-rel vs the fp32
reference on this data (tolerance 2e-2; pure bf16 is 2.0e-3, and extending
fp8 to 512 K measures 1.85e-2 — too close to the gate). fp8 across ALL of K
single-pass would be 2x faster but fails accuracy (5.1e-2), and hi/lo
multi-term fp8 splits cost 1.5x bf16 since DoubleRow does not raise the
column rate — per-instruction K-depth is the only fp8 win.

Schedule per core, two passes over the out-feature halves:
(1) n=0, k-outer/m-inner. Each k-step's x tile and W tile are host-packed
    into ONE bundle = one linear DMA = one completion semaphore, so a k-step
    becomes ready atomically (per-DMA completion order jitters +-1.5us, so
    splitting a k-step across DMAs stalls the PE mid-step and drops the HAM
    clock).
(2) n=1, m-outer/k-inner: everything is SBUF-resident by now (the n=1 W
    prefetches behind the n=0 stream), so each bank runs its 15 matmuls
    (14 bf16 + 1 fp8-DR) back-to-back and drains (copy + store) while the
    next bank computes. The last bank accumulates as two 256-col half-psums
    in different banks (start=True zeroes a whole 2KB bank region) so its
    first half drains during the second half's matmuls, and the two half
    stores ride different engines/queues — the post-last-matmul tail is one
    256-col copy+store.
All input DMAs ride the sync-engine queue in consumption order; y stores ride
the scalar-engine queue so stores never delay loads. Short warmup matmuls
(tiny memset dependency) keep the PE busy from preamble-end until the first
bundle lands, holding the HAM clock gate open so real matmuls start at full
clock.
"""

import numpy as np

TOKENS, IN_F, OUT_F = 4096, 2048, 2048
BLOCK = 32
N_CORES = 8
TG, OG = 4, 2  # token groups x out-feature groups
T_SH = TOKENS // TG  # 1024 tokens per core
O_SH = OUT_F // OG  # 1024 out features per core
P = 128
NFREE = 512  # PSUM bank free dim (fp32)
KT = IN_F // P  # 16 k tiles
KB = 14  # k-tiles computed in bf16; the last 2 run as one fp8 DoubleRow step
MT = T_SH // P  # 8 psum banks
XH = T_SH // 2  # token half (k=0/k=1 head bundles)
N_WARM = 28  # PE clock-gate warmup matmuls (~150ns each, sized to DMA head)

MM_DTYPE = "bfloat16"  # "bfloat16" (fast DMA) or "float32r" (exact-ish)
TRACE = False  # set by test.py to capture an NTFF profile

_nc_cache = {}
_last_result = None  # BassKernelResults of the most recent run (for test.py)


def _build_nc():
    import concourse.mybir as mybir
    import concourse.tile as tile
    from concourse import bacc

    key = MM_DTYPE
    if key in _nc_cache:
        return _nc_cache[key]

    dt_mm = getattr(mybir.dt, MM_DTYPE)
    f32 = mybir.dt.float32
    f8 = mybir.dt.float8e4
    DR = mybir.MatmulPerfMode.DoubleRow

    nc = bacc.Bacc(None, target_bir_lowering=False)
    # Host-pre-blocked inputs (exact SBUF layouts; all DMAs are linear):
    # bn: per-k bf16 bundles [KB][P][T_SH + NFREE] = [x^T k-tile | w n0 k-tile]
    # w1: n=1 bf16 W^T supertile [P][KB][NFREE]
    # b8: fp8 K-segment bundle [P][2][T_SH + NFREE] = [x8 | w8 n0], K-pair
    #     slot i = k-tile 14+i (one DoubleRow matmul contracts all 256 K)
    # w81: fp8 K-segment n=1 W [P][2][NFREE]
    bn = nc.dram_tensor("bn", [KB, P, T_SH + NFREE], dt_mm, kind="ExternalInput")
    w1q = nc.dram_tensor("w1q", [P, KB, NFREE], dt_mm, kind="ExternalInput")
    b8q = nc.dram_tensor("b8q", [P, 2, T_SH + NFREE], f8, kind="ExternalInput")
    w81q = nc.dram_tensor("w81q", [P, 2, NFREE], f8, kind="ExternalInput")
    y = nc.dram_tensor("y", [T_SH, O_SH], f32, kind="ExternalOutput")

    with tile.TileContext(nc) as tc:
        with (
            tc.tile_pool(name="xp", bufs=1) as xp,
            tc.tile_pool(name="wp", bufs=1) as wp,
            tc.tile_pool(name="op", bufs=1) as op,
            tc.tile_pool(name="ps", bufs=1, space="PSUM") as ps,
        ):
            # Warm the PE's HAM clock gate during the initial DMA head wait.
            zt = xp.tile([P, P], dt_mm, tag="warm", name="warm")
            nc.gpsimd.memset(zt[:], 0.0)
            warm_ps = ps.tile([P, NFREE], f32, tag="ps0", name="warm_ps")
            for _ in range(N_WARM):
                nc.tensor.matmul(warm_ps[:, :P], zt[:], zt[:], start=True, stop=True)

            bnt = [None] * KT  # bundle tiles [P, T_SH + NFREE]

            def lhsT(m, k):
                """Stationary x^T slice for bank m, k-tile k."""
                return bnt[k][:, m * P : (m + 1) * P]

            def psums():
                return [
                    ps.tile([P, NFREE], f32, tag=f"ps{m}", name=f"ps{m}")
                    for m in range(MT)
                ]

            # ---- Pass 1: n=0, k-outer/m-inner, bundles streamed JIT ----
            ps0 = psums()
            for k in range(KB):
                t = xp.tile([P, T_SH + NFREE], dt_mm, tag=f"bn{k}", name=f"bn{k}")
                nc.sync.dma_start(t[:], bn[k])
                bnt[k] = t
                for m in range(MT):
                    nc.tensor.matmul(
                        ps0[m][:],
                        lhsT(m, k),
                        t[:, T_SH : T_SH + NFREE],
                        start=(k == 0),
                        stop=False,
                    )
            # fp8 K-segment: one DoubleRow matmul contracts k-tiles 14+15
            b8 = xp.tile([P, 2, T_SH + NFREE], f8, tag="b8", name="b8")
            nc.sync.dma_start(b8[:], b8q[:])
            for m in range(MT):
                nc.tensor.matmul(
                    ps0[m][:],
                    b8[:, :, m * P : (m + 1) * P],
                    b8[:, :, T_SH : T_SH + NFREE],
                    start=False,
                    stop=True,
                    perf_mode=DR,
                )

            # n=1 W: two bf16 prefetches + the fp8 segment W, queued behind
            # the n=0 stream
            w1 = []
            for h in range(2):
                wt = wp.tile([P, KB // 2, NFREE], dt_mm, tag=f"w1_{h}", name=f"w1_{h}")
                nc.sync.dma_start(
                    wt[:], w1q[:, h * (KB // 2) : (h + 1) * (KB // 2), :]
                )
                w1.append(wt)
            w81 = wp.tile([P, 2, NFREE], f8, tag="w81", name="w81")
            nc.sync.dma_start(w81[:], w81q[:])

            for m in range(MT):  # evict n=0 psums; y stores on the scalar queue
                ot = op.tile([P, NFREE], f32, tag=f"o0_{m}", name=f"o0_{m}")
                nc.vector.tensor_copy(ot[:], ps0[m][:])
                nc.scalar.dma_start(y[m * P : (m + 1) * P, 0:NFREE], ot[:])

            # ---- Pass 2: n=1, m-outer/k-inner; each bank drains as it ends ----
            ps1 = psums()
            for m in range(MT):
                ot = op.tile([P, NFREE], f32, tag=f"o1_{m}", name=f"o1_{m}")
                if m == MT - 1:
                    # last bank: accumulate as two 256-col half-psums so the
                    # first half drains while the second computes — the
                    # post-last-matmul chain shrinks to a 256-col copy+store.
                    # start=True zeroes a whole 2KB bank region, so half 1
                    # must NOT share half 0's bank: it reuses bank 0 (m=0's
                    # psum, drained ~20us earlier) instead.
                    for h in range(2):
                        acc = ps1[m] if h == 0 else ps1[0]
                        sl = slice(0, NFREE // 2)
                        hsl = slice(h * (NFREE // 2), (h + 1) * (NFREE // 2))
                        for k in range(KB):
                            nc.tensor.matmul(
                                acc[:, sl],
                                lhsT(m, k),
                                w1[k // (KB // 2)][:, k % (KB // 2), hsl],
                                start=(k == 0),
                                stop=False,
                            )
                        nc.tensor.matmul(
                            acc[:, sl],
                            b8[:, :, m * P : (m + 1) * P],
                            w81[:, :, hsl],
                            start=False,
                            stop=True,
                            perf_mode=DR,
                        )
                        osl = slice(h * (NFREE // 2), (h + 1) * (NFREE // 2))
                        # both halves copy on the (otherwise idle) vector
                        # engine — the halves are staggered 1.6us apart, and
                        # DVE's copy+sem path is ~0.2us faster than Act's
                        nc.vector.tensor_copy(ot[:, osl], acc[:, sl])
                        (nc.scalar if h == 0 else nc.sync).dma_start(
                            y[
                                m * P : (m + 1) * P,
                                NFREE + h * (NFREE // 2) : NFREE
                                + (h + 1) * (NFREE // 2),
                            ],
                            ot[:, osl],
                        )
                else:
                    for k in range(KB):
                        nc.tensor.matmul(
                            ps1[m][:],
                            lhsT(m, k),
                            w1[k // (KB // 2)][:, k % (KB // 2), :],
                            start=(k == 0),
                            stop=False,
                        )
                    nc.tensor.matmul(
                        ps1[m][:],
                        b8[:, :, m * P : (m + 1) * P],
                        w81[:],
                        start=False,
                        stop=True,
                        perf_mode=DR,
                    )
                    nc.vector.tensor_copy(ot[:], ps1[m][:])
                    nc.scalar.dma_start(
                        y[m * P : (m + 1) * P, NFREE : 2 * NFREE], ot[:]
                    )

    nc.compile()
    _nc_cache[key] = nc
    return nc


def _densify_wT(weight_blocks, block_rows, block_cols):
    """Scatter-add the 32x32 blocks into dense W^T [in_features, out_features]."""
    nc_blk = IN_F // BLOCK
    nr_blk = OUT_F // BLOCK
    wcr = np.zeros((nc_blk, nr_blk, BLOCK, BLOCK), np.float32)
    # block b occupies W[32r:32r+32, 32c:32c+32]; W^T gets the transposed block
    np.add.at(
        wcr,
        (block_cols.astype(np.int64), block_rows.astype(np.int64)),
        np.swapaxes(weight_blocks.astype(np.float32, copy=False), 1, 2),
    )
    return np.ascontiguousarray(wcr.transpose(0, 2, 1, 3).reshape(IN_F, OUT_F))


def _mm_np_dtype():
    if MM_DTYPE == "bfloat16":
        import ml_dtypes

        return np.dtype(ml_dtypes.bfloat16)
    return np.dtype(np.float32)


def _pack_core_inputs(xT_sh, wT_sh):
    """Block one core's x^T and W^T shards into the kernel's DMA layouts."""
    import ml_dtypes

    dt = _mm_np_dtype()
    f8 = np.dtype(ml_dtypes.float8_e4m3)
    X = xT_sh.reshape(KT, P, T_SH)  # [k, p, t] fp32
    W = wT_sh.reshape(KT, P, 2, NFREE).transpose(2, 0, 1, 3)  # [n, k, p, o]
    bn = np.concatenate([X[:KB], W[0, :KB]], axis=2).astype(dt)
    w1 = np.ascontiguousarray(W[1, :KB].transpose(1, 0, 2)).astype(dt)
    # fp8 K-segment (k-tiles 14,15 -> DoubleRow pair slot i, global
    # k = KB*128 + i*128 + p for both operands)
    x8 = X[KB:].astype(f8)  # [i, p, t]
    w8 = W[:, KB:].astype(f8)  # [n, i, p, o]
    b8 = np.concatenate([x8, w8[0]], axis=2).transpose(1, 0, 2)  # [p, i, c]
    w81 = w8[1].transpose(1, 0, 2)  # [p, i, o]
    return {
        "bn": np.ascontiguousarray(bn),
        "w1q": w1,
        "b8q": np.ascontiguousarray(b8),
        "w81q": np.ascontiguousarray(w81),
    }


def kernel(x, weight_blocks, block_rows, block_cols):
    global _last_result
    from concourse.bass_utils import run_bass_kernel_spmd

    x = np.asarray(x, dtype=np.float32)
    wT = _densify_wT(
        np.asarray(weight_blocks), np.asarray(block_rows), np.asarray(block_cols)
    )
    xT = np.ascontiguousarray(x.T)

    in_maps = []
    for c in range(N_CORES):
        tg, og = divmod(c, OG)
        in_maps.append(
            _pack_core_inputs(
                xT[:, tg * T_SH : (tg + 1) * T_SH],
                wT[:, og * O_SH : (og + 1) * O_SH],
            )
        )

    nc = _build_nc()
    res = None
    for attempt in range(3):  # transient NRT device errors happen; retry
        try:
            res = run_bass_kernel_spmd(
                nc, in_maps, core_ids=list(range(N_CORES)), trace=TRACE
            )
            break
        except Exception:
            if attempt == 2:
                raise
            import time

            time.sleep(3)
    _last_result = res

    y = np.empty((TOKENS, OUT_F), np.float32)
    for c in range(N_CORES):
        tg, og = divmod(c, OG)
        y[tg * T_SH : (tg + 1) * T_SH, og * O_SH : (og + 1) * O_SH] = res.results[c][
            "y"
        ]
    return y


# revision 38
# speedup vs baseline: 1.0306x; 1.0268x over previous
"""Block-sparse linear y = x @ W^T on 8 Trainium2 NeuronCores.

Strategy: the 32x32 block structure (50% block density, random scatter) is not
exploitable on a 128x128 PE array (any packing at 32-granularity wastes more
PE volume than the ~39% merged density saves), so we densify W^T on the host
(cheap: 8MB of scatter-adds) and run a dense GEMM, sharded 4-way over tokens
x 2-way over out_features (8 cores, no collectives).

The matmuls run in bfloat16 (PE rate is identical to float32r at 1 output
column/cycle, so a pure-bf16 GEMM costs 131072 cycles = 54.6us/core), which
halves input HBM traffic vs fp32r and takes the DMA stream off the PE's
critical path — the fp32r version sat exactly at the 20MB/55us ridge and any
DMA jitter stalled the PE. On top of that, the LAST 256 K-columns run as a
single fp8(e4m3) DoubleRow matmul per psum pass: DoubleRow packs the K-pair
into one partition slot (2 MACs/cell), so one 216ns matmul contracts 256 K —
replacing two bf16 k-steps and cutting PE time ~6% to 51.9us/core. Error is
dominated by the raw-fp8 segment and measures 1.49e-2 max-rel vs the fp32
reference on this data (tolerance 2e-2; pure bf16 is 2.0e-3, and extending
fp8 to 512 K measures 1.85e-2 — too close to the gate). fp8 across ALL of K
single-pass would be 2x faster but fails accuracy (5.1e-2), and hi/lo
multi-term fp8 splits cost 1.5x bf16 since DoubleRow does not raise the
column rate — per-instruction K-depth is the only fp8 win.

Schedule per core, two passes over the out-feature halves:
(1) n=0, k-outer/m-inner. Each k-step's x tile and W tile are host-packed
    into ONE bundle = one linear DMA = one completion semaphore, so a k-step
    becomes ready atomically (per-DMA completion order jitters +-1.5us, so
    splitting a k-step across DMAs stalls the PE mid-step and drops the HAM
    clock).
(2) n=1, m-outer/k-inner: everything is SBUF-resident by now (the n=1 W
    prefetches behind the n=0 stream), so each bank runs its 15 matmuls
    (14 bf16 + 1 fp8-DR) back-to-back and drains (copy + store) while the
    next bank computes. The last bank accumulates as two 256-col half-psums
    in different banks (start=True zeroes a whole 2KB bank region) so its
    first half drains during the second half's matmuls, and the two half
    stores ride different engines/queues — the post-last-matmul tail is one
    256-col copy+store.
All input DMAs ride the sync-engine queue in consumption order; y stores ride
the scalar-engine queue so stores never delay loads. Short warmup matmuls
(tiny memset dependency) keep the PE busy from preamble-end until the first
bundle lands, holding the HAM clock gate open so real matmuls start at full
clock.
"""

import numpy as np

TOKENS, IN_F, OUT_F = 4096, 2048, 2048
BLOCK = 32
N_CORES = 8
TG, OG = 4, 2  # token groups x out-feature groups
T_SH = TOKENS // TG  # 1024 tokens per core
O_SH = OUT_F // OG  # 1024 out features per core
P = 128
NFREE = 512  # PSUM bank free dim (fp32)
KT = IN_F // P  # 16 k tiles
KB = 14  # k-tiles computed in bf16; the last 2 run as one fp8 DoubleRow step
MT = T_SH // P  # 8 psum banks
XH = T_SH // 2  # token half (k=0/k=1 head bundles)
N_WARM = 26  # PE clock-gate warmup matmuls (~150ns each, sized to DMA head)

MM_DTYPE = "bfloat16"  # "bfloat16" (fast DMA) or "float32r" (exact-ish)
TRACE = False  # set by test.py to capture an NTFF profile

_nc_cache = {}
_last_result = None  # BassKernelResults of the most recent run (for test.py)


def _build_nc():
    import concourse.mybir as mybir
    import concourse.tile as tile
    from concourse import bacc

    key = MM_DTYPE
    if key in _nc_cache:
        return _nc_cache[key]

    dt_mm = getattr(mybir.dt, MM_DTYPE)
    f32 = mybir.dt.float32
    f8 = mybir.dt.float8e4
    DR = mybir.MatmulPerfMode.DoubleRow

    nc = bacc.Bacc(None, target_bir_lowering=False)
    # Host-pre-blocked inputs (exact SBUF layouts; all DMAs are linear):
    # bn: per-k bf16 bundles [KB][P][T_SH + NFREE] = [x^T k-tile | w n0 k-tile]
    # w1: n=1 bf16 W^T supertile [P][KB][NFREE]
    # b8: fp8 K-segment bundle [P][2][T_SH + NFREE] = [x8 | w8 n0], K-pair
    #     slot i = k-tile 14+i (one DoubleRow matmul contracts all 256 K)
    # w81: fp8 K-segment n=1 W [P][2][NFREE]
    bn = nc.dram_tensor("bn", [KB, P, T_SH + NFREE], dt_mm, kind="ExternalInput")
    w1q = nc.dram_tensor("w1q", [P, KB, NFREE], dt_mm, kind="ExternalInput")
    b8q = nc.dram_tensor("b8q", [P, 2, T_SH + NFREE], f8, kind="ExternalInput")
    w81q = nc.dram_tensor("w81q", [P, 2, NFREE], f8, kind="ExternalInput")
    y = nc.dram_tensor("y", [T_SH, O_SH], f32, kind="ExternalOutput")

    with tile.TileContext(nc) as tc:
        with (
            tc.tile_pool(name="xp", bufs=1) as xp,
            tc.tile_pool(name="wp", bufs=1) as wp,
            tc.tile_pool(name="op", bufs=1) as op,
            tc.tile_pool(name="ps", bufs=1, space="PSUM") as ps,
        ):
            # Warm the PE's HAM clock gate during the initial DMA head wait.
            zt = xp.tile([P, P], dt_mm, tag="warm", name="warm")
            nc.gpsimd.memset(zt[:], 0.0)
            warm_ps = ps.tile([P, NFREE], f32, tag="ps0", name="warm_ps")
            for _ in range(N_WARM):
                nc.tensor.matmul(warm_ps[:, :P], zt[:], zt[:], start=True, stop=True)

            bnt = [None] * KT  # bundle tiles [P, T_SH + NFREE]

            def lhsT(m, k):
                """Stationary x^T slice for bank m, k-tile k."""
                return bnt[k][:, m * P : (m + 1) * P]

            def psums():
                return [
                    ps.tile([P, NFREE], f32, tag=f"ps{m}", name=f"ps{m}")
                    for m in range(MT)
                ]

            # ---- Pass 1: n=0, k-outer/m-inner, bundles streamed JIT ----
            ps0 = psums()
            for k in range(KB):
                t = xp.tile([P, T_SH + NFREE], dt_mm, tag=f"bn{k}", name=f"bn{k}")
                nc.sync.dma_start(t[:], bn[k])
                bnt[k] = t
                for m in range(MT):
                    nc.tensor.matmul(
                        ps0[m][:],
                        lhsT(m, k),
                        t[:, T_SH : T_SH + NFREE],
                        start=(k == 0),
                        stop=False,
                    )
            # fp8 K-segment: one DoubleRow matmul contracts k-tiles 14+15
            b8 = xp.tile([P, 2, T_SH + NFREE], f8, tag="b8", name="b8")
            nc.sync.dma_start(b8[:], b8q[:])
            for m in range(MT):
                nc.tensor.matmul(
                    ps0[m][:],
                    b8[:, :, m * P : (m + 1) * P],
                    b8[:, :, T_SH : T_SH + NFREE],
                    start=False,
                    stop=True,
                    perf_mode=DR,
                )

            # n=1 W: two bf16 prefetches + the fp8 segment W, queued behind
            # the n=0 stream
            w1 = []
            for h in range(2):
                wt = wp.tile([P, KB // 2, NFREE], dt_mm, tag=f"w1_{h}", name=f"w1_{h}")
                nc.sync.dma_start(
                    wt[:], w1q[:, h * (KB // 2) : (h + 1) * (KB // 2), :]
                )
                w1.append(wt)
            w81 = wp.tile([P, 2, NFREE], f8, tag="w81", name="w81")
            nc.sync.dma_start(w81[:], w81q[:])

            for m in range(MT):  # evict n=0 psums; y stores on the scalar queue
                ot = op.tile([P, NFREE], f32, tag=f"o0_{m}", name=f"o0_{m}")
                nc.vector.tensor_copy(ot[:], ps0[m][:])
                nc.scalar.dma_start(y[m * P : (m + 1) * P, 0:NFREE], ot[:])

            # ---- Pass 2: n=1, m-outer/k-inner; each bank drains as it ends ----
            ps1 = psums()
            for m in range(MT):
                ot = op.tile([P, NFREE], f32, tag=f"o1_{m}", name=f"o1_{m}")
                if m == MT - 1:
                    # last bank: accumulate as two 256-col half-psums so the
                    # first half drains while the second computes — the
                    # post-last-matmul chain shrinks to a 256-col copy+store.
                    # start=True zeroes a whole 2KB bank region, so half 1
                    # must NOT share half 0's bank: it reuses bank 0 (m=0's
                    # psum, drained ~20us earlier) instead.
                    for h in range(2):
                        acc = ps1[m] if h == 0 else ps1[0]
                        sl = slice(0, NFREE // 2)
                        hsl = slice(h * (NFREE // 2), (h + 1) * (NFREE // 2))
                        for k in range(KB):
                            nc.tensor.matmul(
                                acc[:, sl],
                                lhsT(m, k),
                                w1[k // (KB // 2)][:, k % (KB // 2), hsl],
                                start=(k == 0),
                                stop=False,
                            )
                        nc.tensor.matmul(
                            acc[:, sl],
                            b8[:, :, m * P : (m + 1) * P],
                            w81[:, :, hsl],
                            start=False,
                            stop=True,
                            perf_mode=DR,
                        )
                        osl = slice(h * (NFREE // 2), (h + 1) * (NFREE // 2))
                        # both halves copy on the (otherwise idle) vector
                        # engine — the halves are staggered 1.6us apart, and
                        # DVE's copy+sem path is ~0.2us faster than Act's
                        nc.vector.tensor_copy(ot[:, osl], acc[:, sl])
                        (nc.scalar if h == 0 else nc.sync).dma_start(
                            y[
                                m * P : (m + 1) * P,
                                NFREE + h * (NFREE // 2) : NFREE
                                + (h + 1) * (NFREE // 2),
                            ],
                            ot[:, osl],
                        )
                else:
                    for k in range(KB):
                        nc.tensor.matmul(
                            ps1[m][:],
                            lhsT(m, k),
                            w1[k // (KB // 2)][:, k % (KB // 2), :],
                            start=(k == 0),
                            stop=False,
                        )
                    nc.tensor.matmul(
                        ps1[m][:],
                        b8[:, :, m * P : (m + 1) * P],
                        w81[:],
                        start=False,
                        stop=True,
                        perf_mode=DR,
                    )
                    nc.vector.tensor_copy(ot[:], ps1[m][:])
                    nc.scalar.dma_start(
                        y[m * P : (m + 1) * P, NFREE : 2 * NFREE], ot[:]
                    )

    nc.compile()
    _nc_cache[key] = nc
    return nc


def _densify_wT(weight_blocks, block_rows, block_cols):
    """Scatter-add the 32x32 blocks into dense W^T [in_features, out_features]."""
    nc_blk = IN_F // BLOCK
    nr_blk = OUT_F // BLOCK
    wcr = np.zeros((nc_blk, nr_blk, BLOCK, BLOCK), np.float32)
    # block b occupies W[32r:32r+32, 32c:32c+32]; W^T gets the transposed block
    np.add.at(
        wcr,
        (block_cols.astype(np.int64), block_rows.astype(np.int64)),
        np.swapaxes(weight_blocks.astype(np.float32, copy=False), 1, 2),
    )
    return np.ascontiguousarray(wcr.transpose(0, 2, 1, 3).reshape(IN_F, OUT_F))


def _mm_np_dtype():
    if MM_DTYPE == "bfloat16":
        import ml_dtypes

        return np.dtype(ml_dtypes.bfloat16)
    return np.dtype(np.float32)


def _pack_core_inputs(xT_sh, wT_sh):
    """Block one core's x^T and W^T shards into the kernel's DMA layouts."""
    import ml_dtypes

    dt = _mm_np_dtype()
    f8 = np.dtype(ml_dtypes.float8_e4m3)
    X = xT_sh.reshape(KT, P, T_SH)  # [k, p, t] fp32
    W = wT_sh.reshape(KT, P, 2, NFREE).transpose(2, 0, 1, 3)  # [n, k, p, o]
    bn = np.concatenate([X[:KB], W[0, :KB]], axis=2).astype(dt)
    w1 = np.ascontiguousarray(W[1, :KB].transpose(1, 0, 2)).astype(dt)
    # fp8 K-segment (k-tiles 14,15 -> DoubleRow pair slot i, global
    # k = KB*128 + i*128 + p for both operands)
    x8 = X[KB:].astype(f8)  # [i, p, t]
    w8 = W[:, KB:].astype(f8)  # [n, i, p, o]
    b8 = np.concatenate([x8, w8[0]], axis=2).transpose(1, 0, 2)  # [p, i, c]
    w81 = w8[1].transpose(1, 0, 2)  # [p, i, o]
    return {
        "bn": np.ascontiguousarray(bn),
        "w1q": w1,
        "b8q": np.ascontiguousarray(b8),
        "w81q": np.ascontiguousarray(w81),
    }


def kernel(x, weight_blocks, block_rows, block_cols):
    global _last_result
    from concourse.bass_utils import run_bass_kernel_spmd

    x = np.asarray(x, dtype=np.float32)
    wT = _densify_wT(
        np.asarray(weight_blocks), np.asarray(block_rows), np.asarray(block_cols)
    )
    xT = np.ascontiguousarray(x.T)

    in_maps = []
    for c in range(N_CORES):
        tg, og = divmod(c, OG)
        in_maps.append(
            _pack_core_inputs(
                xT[:, tg * T_SH : (tg + 1) * T_SH],
                wT[:, og * O_SH : (og + 1) * O_SH],
            )
        )

    nc = _build_nc()
    res = None
    for attempt in range(3):  # transient NRT device errors happen; retry
        try:
            res = run_bass_kernel_spmd(
                nc, in_maps, core_ids=list(range(N_CORES)), trace=TRACE
            )
            break
        except Exception:
            if attempt == 2:
                raise
            import time

            time.sleep(3)
    _last_result = res

    y = np.empty((TOKENS, OUT_F), np.float32)
    for c in range(N_CORES):
        tg, og = divmod(c, OG)
        y[tg * T_SH : (tg + 1) * T_SH, og * O_SH : (og + 1) * O_SH] = res.results[c][
            "y"
        ]
    return y
